# revision 15
# baseline (speedup 1.0000x reference)
"""Multi-head self-attention (B=2, N=2048, D=1024, H=16, Dh=64) on 8 TRN2 NeuronCores.

Sharding: core c handles batch b = c // 4 and head group g = c % 4 (heads 4g..4g+3).
Each core produces a partial [D, N] bf16 output (transposed); host sums the 4
head-group partials per batch, transposes, and adds b_out.

Schedule (single ACT-paced stream; all matmuls bf16, fp32 PSUM):
  A1a  q,k projection for head pair 0, d-major so compute tracks the x DMA
       stream (dual hardware DMA queues: Sync + Activation engines).
  B    128 iterations of [scores pair (row-tiled, concurrent) -> exp -> attn@V].
       attn@V is TRANSPOSED: out2[q,dh] = pt_chunk.T @ v (stationary = attn
       weights, FWL), 8 accumulation groups packed into 2 PSUM banks using
       explicit start=False overwrites after one bank-clearing start=True.
       Softmax denominators land per-partition -> reciprocal + tensor_scalar
       normalization, PE transpose back to [c,q] for the out-projection.
       PE slack under the ACT exp pacing is filled with the v projection (A2),
       pair-1 q,k projection (A1b), and the out-projection (C).
"""
import sys
import numpy as np

for _p in ("/opt/trn_rl_repo", "/root/.axon_site/_ro/trn_rl_repo"):
    if _p not in sys.path:
        sys.path.append(_p)

import ml_dtypes

import concourse.bass as bass
import concourse.bacc as bacc
import concourse.tile as tile
from concourse import mybir
from concourse.bass_utils import run_bass_kernel_spmd

F32 = mybir.dt.float32
BF16 = mybir.dt.bfloat16
EXP = mybir.ActivationFunctionType.Exp
NP_BF16 = ml_dtypes.bfloat16

B, S, D = 2, 2048, 1024
H, DH = 16, 64
HL = 4            # heads per core (local)
CQK = 512         # q+k channels per core (2*HL*DH)
CV = 256          # v channels per core (HL*DH)
ND = D // 128     # 8 d-tiles
NKT = S // 128    # 16 key tiles
NQC = S // 512    # 4 query chunks of 512


def build_kernel() -> "bass.Bass":
    nc = bacc.Bacc(None, target_bir_lowering=False, debug=False)

    xT = nc.dram_tensor("xT", [D, S], BF16, kind="ExternalInput")
    wqk = nc.dram_tensor("wqk", [D, CQK], BF16, kind="ExternalInput")
    bqk = nc.dram_tensor("bqk", [128, CQK // 128], F32, kind="ExternalInput")
    wv = nc.dram_tensor("wv", [D, CV], BF16, kind="ExternalInput")
    bvb = nc.dram_tensor("bvb", [128, CV], F32, kind="ExternalInput")
    wout = nc.dram_tensor("wout", [CV, D], BF16, kind="ExternalInput")
    ident = nc.dram_tensor("ident", [128, 128], BF16, kind="ExternalInput")
    outT = nc.dram_tensor("outT", [D, S], BF16, kind="ExternalOutput")

    xT_r = xT.rearrange("(t p) s -> t p s", p=128)        # [8, 128, 2048]
    wqk_r = wqk.rearrange("(t p) c -> t p c", p=128)      # [8, 128, 512]
    wv_r = wv.rearrange("(t p) c -> t p c", p=128)        # [8, 128, 256]
    wout_r = wout.rearrange("(t p) n -> t p n", p=128)    # [2, 128, 1024]
    outT_r = outT.rearrange("(t p) s -> t p s", p=128)    # [8, 128, 2048]

    with tile.TileContext(nc) as tc:
        with tc.tile_pool(name="persist", bufs=1) as persist:
            qkt_s = persist.tile([128, 4, S], BF16)           # q0 q1 k0 k1
            v_s = persist.tile([128, NKT, HL, DH + 1], BF16)  # V + ones col
            at_s = persist.tile([128, 2, S], BF16)            # attn out^T (c, q)
            wout_s = persist.tile([128, 2, D], BF16)
            bqk_s = persist.tile([128, CQK // 128], F32)
            bvb_s = persist.tile([128, CV], F32)
            ident_s = persist.tile([128, 128], BF16)
            scratch = persist.tile([1, 8], F32)

            # pre-warm the ACT exp table while input DMAs run
            nc.vector.memset(scratch[:], 0.0)
            nc.scalar.activation(scratch[:], scratch[:], EXP)
            nc.vector.memset(v_s[:, :, :, DH:DH + 1], 1.0)

            pha_ctx = [tc.tile_pool(name="phA", bufs=1),
                       tc.tile_pool(name="phA_ps", bufs=1, space="PSUM")]
            pha, pps = [c.__enter__() for c in pha_ctx]
            xt_s = pha.tile([128, ND, S], BF16)
            wqk_s = pha.tile([128, ND, CQK], BF16)
            wv_s = pha.tile([128, ND, CV], BF16)

            # ---- input DMA: alternate the two hardware queues per d-tile ----
            for d in range(ND):
                qa, qb_ = (nc.sync, nc.scalar) if d % 2 == 0 else (nc.scalar, nc.sync)
                qa.dma_start(out=wqk_s[:, d, :], in_=wqk_r[d])
                qb_.dma_start(out=xt_s[:, d, :], in_=xT_r[d])
                if d == 0:
                    nc.sync.dma_start(out=bqk_s[:], in_=bqk[:])
                if d == 3:
                    nc.scalar.dma_start(out=bvb_s[:], in_=bvb[:])
            for d in range(ND):
                (nc.sync if d % 2 == 0 else nc.scalar).dma_start(
                    out=wv_s[:, d, :], in_=wv_r[d])
            nc.scalar.dma_start(out=ident_s[:], in_=ident[:])
            for t in range(2):
                nc.sync.dma_start(out=wout_s[:, t, :], in_=wout_r[t])

            # ---- A1a: q,k projection for pair 0 (m=0 q, m=2 k), d-major ----
            psA = {(m, n): pps.tile([128, 512], F32, tag=f"a{m}{n}", name=f"a{m}{n}")
                   for m in (0, 2) for n in range(NQC)}
            for d in range(ND - 1):
                for m in (0, 2):
                    lhsT = wqk_s[:, d, m * 128:(m + 1) * 128]
                    for n in range(NQC):
                        nc.tensor.matmul(psA[(m, n)][:], lhsT,
                                         xt_s[:, d, n * 512:(n + 1) * 512],
                                         start=(d == 0), stop=False)
            # last d-tile: finish chain (m, n) then bias-add it immediately so
            # the first scores don't wait for all eight serial DVE adds
            d = ND - 1
            for n in range(NQC):
                for m in (0, 2):
                    nc.tensor.matmul(psA[(m, n)][:],
                                     wqk_s[:, d, m * 128:(m + 1) * 128],
                                     xt_s[:, d, n * 512:(n + 1) * 512],
                                     start=False, stop=True)
                for m in (0, 2):
                    nc.vector.tensor_scalar_add(
                        qkt_s[:, m, n * 512:(n + 1) * 512], psA[(m, n)][:],
                        bqk_s[:, m:m + 1])

            pha_ctx[1].__exit__(None, None, None)   # free the 8 A psum banks

            # ---- B pools: sAB x2 = 4 banks, out2 (pA,pB) = 2, tp = 1, aux = 1
            b_ctx = [tc.tile_pool(name="ptp", bufs=18),
                     tc.tile_pool(name="smallB", bufs=4),
                     tc.tile_pool(name="a2n", bufs=3),
                     tc.tile_pool(name="stage", bufs=4),
                     tc.tile_pool(name="phB_s", bufs=2, space="PSUM"),
                     tc.tile_pool(name="phB_av", bufs=1, space="PSUM"),
                     tc.tile_pool(name="phB_tp", bufs=1, space="PSUM"),
                     tc.tile_pool(name="aux_ps", bufs=1, space="PSUM")]
            ptp, small, a2np, stage, psb, psav, pstp, aux = [
                c.__enter__() for c in b_ctx]

            # ---------------- emission helpers ----------------
            a2_tiles_done = [0]     # v tiles whose A2 chain is fully emitted

            def filler_gen():
                # A2: v projection, one key tile per chain (aux bank)
                for st in range(NKT):
                    ps = aux.tile([128, CV], F32, tag="px", name="psv")
                    for d in range(ND):
                        nc.tensor.matmul(ps[:],
                                         xt_s[:, d, st * 128:(st + 1) * 128],
                                         wv_s[:, d, :],
                                         start=(d == 0), stop=(d == ND - 1))
                        yield
                    nc.vector.tensor_tensor(
                        out=v_s[:, st, :, 0:DH],
                        in0=ps[:].rearrange("p (h c) -> p h c", h=HL),
                        in1=bvb_s[:].rearrange("p (h c) -> p h c", h=HL),
                        op=mybir.AluOpType.add)
                    a2_tiles_done[0] = st + 1
                    yield
                # A1b: pair-1 q,k projection, ordered by first consumption
                for m, n in ((1, 0), (3, 0), (3, 1), (3, 2), (3, 3),
                             (1, 1), (1, 2), (1, 3)):
                    ps = aux.tile([128, 512], F32, tag="px", name="pqk")
                    for d in range(ND):
                        nc.tensor.matmul(ps[:],
                                         wqk_s[:, d, m * 128:(m + 1) * 128],
                                         xt_s[:, d, n * 512:(n + 1) * 512],
                                         start=(d == 0), stop=(d == ND - 1))
                        yield
                    nc.vector.tensor_scalar_add(
                        qkt_s[:, m, n * 512:(n + 1) * 512], ps[:],
                        bqk_s[:, m:m + 1])
                    yield

            def norm_gen(p, qb, pA, pB):
                """Normalize out2 by softmax denominator, transpose to at_s."""
                for qc in range(NQC):
                    a2n = a2np.tile([128, 128], BF16, tag="a2n", name="a2n")
                    for h, pX in ((0, pA), (1, pB)):
                        rr = small.tile([128, 1], F32, tag="rr", name="rr")
                        nc.vector.reciprocal(rr[:], pX[:, qc, DH:DH + 1])
                        nc.vector.tensor_scalar_mul(
                            a2n[:, h * 64:(h + 1) * 64], pX[:, qc, 0:DH], rr[:])
                        yield
                    tp = pstp.tile([128, 128], BF16, tag="tp", name="tp")
                    nc.tensor.transpose(tp[:], a2n[:], ident_s[:])
                    nc.vector.tensor_copy(
                        out=at_s[:, p, qb * 512 + qc * 128:qb * 512 + (qc + 1) * 128],
                        in_=tp[:])
                    yield

            def c_gen(qc):
                """Out-projection for query block qc (po alternates aux/tp banks)."""
                qg = slice(qc * 512, (qc + 1) * 512)
                for nt in range(ND):
                    pool, tag = (aux, "px") if nt % 2 == 0 else (pstp, "tp")
                    po = pool.tile([128, 512], F32, tag=tag, name=f"po{nt % 2}")
                    for ct in range(2):
                        nc.tensor.matmul(po[:],
                                         wout_s[:, ct, nt * 128:(nt + 1) * 128],
                                         at_s[:, ct, qg],
                                         start=(ct == 0), stop=(ct == 1))
                    o = stage.tile([128, 512], BF16, tag="o", name="o")
                    nc.vector.tensor_copy(out=o[:], in_=po[:])
                    nc.sync.dma_start(out=outT_r[nt][:, qg], in_=o[:])
                    yield

            def emit_scores_exp(p, qb, t):
                qt = qkt_s[:, p, :]
                kt = qkt_s[:, 2 + p, :]
                qs = slice(qb * 512, (qb + 1) * 512)
                sAB = psb.tile([128, 1024], F32, tag="sAB", name="sAB")
                nc.tensor.matmul(sAB[:, 0:512],
                                 kt[0:64, t * 128:(t + 1) * 128],
                                 qt[0:64, qs], start=True, stop=True,
                                 tile_position=(0, 0))
                nc.tensor.matmul(sAB[:, 512:1024],
                                 kt[64:128, t * 128:(t + 1) * 128],
                                 qt[64:128, qs], start=True, stop=True,
                                 tile_position=(64, 0))
                pt = ptp.tile([128, 1024], BF16, tag="pt", name="pt")
                nc.scalar.activation(pt[:], sAB[:], EXP)
                return pt

            def emit_avt(p, t, pt, pA, pB):
                # transposed attn@V: out2[q,65] += pt_chunk.T @ [v | 1]
                # one bank-clearing start per bank per qb round (qc==0, t==0)
                for h, pX in ((0, pA), (1, pB)):
                    for qc in range(NQC):
                        nc.tensor.matmul(
                            pX[:, qc, :],
                            pt[:, h * 512 + qc * 128:h * 512 + (qc + 1) * 128],
                            v_s[:, t, 2 * p + h, :],
                            start=(t == 0 and qc == 0), stop=(t == NKT - 1),
                            skip_group_check=True)

            # ---------------- B driver ----------------
            # Emission-order invariants (deps are inserted at emission time):
            #   AVT(block k, t) only after: A2 chain for v[t] emitted, AND
            #     norm(block k-1) fully emitted (pA/pB bank WAR direction).
            #   norm(block k) only after all 16 AVT groups of block k emitted.
            fill = filler_gen()
            fill_done = [False]

            _DONE = object()

            def drive(gen, k):
                n = 0
                for _ in range(k):
                    if next(gen, _DONE) is _DONE:
                        return n
                    n += 1
                return n

            gens = []            # FIFO of [kind, gen, key] for norms + c chains
            avt_pending = []     # (key, t, pt, pA, pB, p)
            avt_done = [0] * 8   # AVT groups emitted per block
            tavt = [0]           # total AVT groups emitted
            norm_done = [False] * 8

            def drive_gens(budget):
                while budget > 0 and gens:
                    kind, g, key = gens[0]
                    if kind.startswith("norm") and avt_done[key] < NKT:
                        return          # gate: block's AVT not fully emitted
                    if next(g, _DONE) is _DONE:
                        gens.pop(0)
                        if kind.startswith("norm"):
                            norm_done[key] = True
                            if kind == "norm1":
                                gens.append(["c", c_gen(key - NQC), key])
                    else:
                        budget -= 1

            def pump_avt(quota):
                while avt_pending and quota > 0:
                    key, tt, ptt, pa, pb, pp = avt_pending[0]
                    if a2_tiles_done[0] <= tt:
                        break
                    if key > 0 and not norm_done[key - 1]:
                        break
                    avt_pending.pop(0)
                    emit_avt(pp, tt, ptt, pa, pb)
                    avt_done[key] += 1
                    tavt[0] += 1
                    quota -= 1

            for p in range(2):
                for qb in range(NQC):
                    key = p * NQC + qb
                    pA = psav.tile([128, NQC, DH + 1], F32, tag="pA", name="pA")
                    pB = psav.tile([128, NQC, DH + 1], F32, tag="pB", name="pB")
                    for t in range(NKT):
                        i = key * NKT + t
                        # pt pool rotation safety: allocation #i reuses the
                        # slot of #i-16, whose AVT readers must be emitted
                        guard = 0
                        while tavt[0] < i - 13 and guard < 1000:
                            pump_avt(4)
                            drive_gens(8)
                            guard += 1
                        pt = emit_scores_exp(p, qb, t)
                        avt_pending.append((key, t, pt, pA, pB, p))
                        pump_avt(2 if len(avt_pending) > 6 else 1)
                        drive_gens(2)
                        if not fill_done[0]:
                            k = 5 if i < 16 else (4 if i < 32 else 2)
                            if drive(fill, k) < k:
                                fill_done[0] = True
                    if key == 2 * NQC - 1:
                        # tail: drain AVT backlog + all pending generators,
                        # then emit norm(7) interleaved with the final
                        # out-projection at 128-column granularity, using the
                        # now-dead sAB banks for the psum tiles
                        while avt_pending:
                            n0 = len(avt_pending)
                            pump_avt(n0)
                            if len(avt_pending) == n0:
                                drive_gens(16)
                        drive_gens(10 ** 6)
                        for qc in range(NQC):
                            a2n = a2np.tile([128, 128], BF16, tag="a2n", name="a2n")
                            for h, pX in ((0, pA), (1, pB)):
                                rr = small.tile([128, 1], F32, tag="rr", name="rr")
                                nc.vector.reciprocal(rr[:], pX[:, qc, DH:DH + 1])
                                nc.vector.tensor_scalar_mul(
                                    a2n[:, h * 64:(h + 1) * 64],
                                    pX[:, qc, 0:DH], rr[:])
                            tp = pstp.tile([128, 128], BF16, tag="tp", name="tp")
                            nc.tensor.transpose(tp[:], a2n[:], ident_s[:])
                            q0 = qb * 512 + qc * 128
                            nc.vector.tensor_copy(
                                out=at_s[:, p, q0:q0 + 128], in_=tp[:])
                            for nt in range(ND):
                                po = psb.tile([128, 128], F32, tag="sAB",
                                              name="po3")
                                for ct in range(2):
                                    nc.tensor.matmul(
                                        po[:],
                                        wout_s[:, ct, nt * 128:(nt + 1) * 128],
                                        at_s[:, ct, q0:q0 + 128],
                                        start=(ct == 0), stop=(ct == 1))
                                o = stage.tile([128, 128], BF16, tag="o3",
                                               name="o3")
                                nc.vector.tensor_copy(out=o[:], in_=po[:])
                                nc.sync.dma_start(
                                    out=outT_r[nt][:, q0:q0 + 128], in_=o[:])
                    else:
                        gens.append(["norm1" if p == 1 else "norm0",
                                     norm_gen(p, qb, pA, pB), key])
                if p == 0:
                    # pair-1 scores need A1b complete: force-drain fillers
                    while drive(fill, 64) == 64:
                        pass
                    fill_done[0] = True

            drive_gens(10 ** 6)

            for c in reversed(b_ctx):
                c.__exit__(None, None, None)
            pha_ctx[0].__exit__(None, None, None)
    nc.compile()
    return nc


def shard_inputs(x, W_qkv, b_qkv, W_out, b_out=None):
    """Build the 8 per-core input maps. Core c: batch c//4, head group c%4."""
    in_maps = []
    scale = 1.0 / np.sqrt(np.float32(DH))
    ident = np.eye(128, dtype=NP_BF16)
    for c in range(8):
        b, g = divmod(c, 4)
        cs = slice(g * 256, g * 256 + 256)
        xTc = np.ascontiguousarray(x[b].T)                       # [D, S]
        wq = W_qkv[:, 0:D][:, cs] * scale                        # [D, 256]
        wk = W_qkv[:, D:2 * D][:, cs]
        wqkc = np.ascontiguousarray(np.concatenate([wq, wk], axis=1))  # [D, 512]
        bq = b_qkv[0:D][cs] * scale
        bk = b_qkv[D:2 * D][cs]
        bqkc = np.concatenate([bq, bk]).reshape(CQK // 128, 128).T     # [128, 4]
        bqkc = np.ascontiguousarray(bqkc)
        wvc = np.ascontiguousarray(W_qkv[:, 2 * D:3 * D][:, cs])       # [D, 256]
        bvbc = np.ascontiguousarray(
            np.broadcast_to(b_qkv[2 * D:3 * D][cs], (128, CV)))        # [128, 256]
        woutc = np.ascontiguousarray(W_out[cs, :])                     # [256, D]
        in_maps.append({
            "xT": xTc.astype(NP_BF16),
            "wqk": wqkc.astype(NP_BF16),
            "bqk": bqkc.astype(np.float32),
            "wv": wvc.astype(NP_BF16),
            "bvb": bvbc.astype(np.float32),
            "wout": woutc.astype(NP_BF16),
            "ident": ident,
        })
    return in_maps


_NC_CACHE = []


def _get_nc():
    if not _NC_CACHE:
        _NC_CACHE.append(build_kernel())
    return _NC_CACHE[0]


def run_sharded(in_maps, **kwargs):
    nc = _get_nc()
    return run_bass_kernel_spmd(nc, in_maps, core_ids=list(range(8)), **kwargs)


def gather_output(results, b_out):
    out = np.empty((B, S, D), dtype=np.float32)
    for b in range(B):
        acc = results[4 * b]["outT"].astype(np.float32)
        for g in range(1, 4):
            acc = acc + results[4 * b + g]["outT"].astype(np.float32)
        out[b] = acc.T + b_out[None, :]
    return out


def kernel(x, W_qkv, b_qkv, W_out, b_out):
    x = np.asarray(x, dtype=np.float32)
    W_qkv = np.asarray(W_qkv, dtype=np.float32)
    b_qkv = np.asarray(b_qkv, dtype=np.float32)
    W_out = np.asarray(W_out, dtype=np.float32)
    b_out = np.asarray(b_out, dtype=np.float32)
    in_maps = shard_inputs(x=x, W_qkv=W_qkv, b_qkv=b_qkv, W_out=W_out, b_out=b_out)
    res = run_sharded(in_maps)
    return gather_output(res.results, b_out)
